# revision 27
# baseline (speedup 1.0000x reference)
"""RBF-kernel SVM decision function on 8 TRN2 NeuronCores.

out[i] = sum_j alphas[j] * exp(-GAMMA * ||x[i] - supports[j]||^2)

Factorization: out_i = u_i * sum_j sgn_j * exp(e_ij),
  e_ij = (x_i/32).s_j + (ln|a_j| - g|s_j|^2)   [PSUM via one fp8 DoubleRow
         matmul; see row scheme in _prepare]
  u_i  = exp(-g|x_i|^2)                        [applied at the end, per i]

PE: fp8 e4m3 DoubleRow matmul built from range-scaled hi/lo splits
(A1=e4m3(x/4), A2=e4m3(16(x/4-A1)); B1=e4m3(s/8), B2=e4m3(16(s/8-B1));
w = A1.B1 + (A1/16).B2 + A2.(B1/16), jterm via 3 rows J1+J2/16+J3/256;
196 live rows of [128 partitions, 2 sub-rows]). Exponent error ~6e-4 rms.

Drain structure: PSUM holds two 2048-col windows (ping-pong); each window
is [0:Q) for ACT and [Q:2048) for DVE:
  - ACT: one ACTIVATE(Exp, accum_out) per window; the host permutation
    makes each window's ACT piece sign-pure (windows 0,1 take the
    largest-|a| positives, windows 2,3 the largest-|a| negatives, so ACT
    covers the global top-|a| columns of both signs).
  - DVE: one Schraudolph TENSOR_SCALAR per window producing uint16 codes
    (round(1024*(log2e*e + 23 - SIGMA))); the code bit pattern read as
    fp16 is ~exp(e)*2^8. Negative-alpha DVE columns get their SIGN baked
    into the matmul: the host adds 32768/A_SC to their jterm so the code
    lands with the fp16 sign bit set -- no flip splits, one piece/window.
The codes fold 4x before a single CACHE_REDUCE: windows 0-2 fold on the
otherwise-idle Pool engine (fires per window, right after its codes
land); window 3 folds on the DVE (fp16 2x mode, short tail). The CR and
the tiny final combine are emitted one i-tile late (and at low scheduler
priority) so they never block the DVE FIFO ahead of the next tile's
pieces.

Steady state is four-way balanced: Scalar ~7.0us/i-tile (4 ACTIVATE +
4 READ_ACCUMULATOR), Vector ~6.9 (4 TS + CR + w3 folds), Pool ~6.7
(6 folds + combine), PE ~6.6 (16 DoubleRow MMs; per-MM LDWEIGHTS is
compiler-mandated). Further Q rebalancing moves time between Scalar and
Vector symmetrically -- Q=1136 sits at the crossover.
"""

import os
import sys

for p in ("/opt/trn_rl_repo",):
    if p not in sys.path:
        sys.path.insert(0, p)

import numpy as np
import ml_dtypes

import concourse.bass as bass
import concourse.tile as tile
from concourse import bacc, mybir
from concourse.bass_utils import run_bass_kernel_spmd

N_CORES = 8
N = 16384
M = 8192
F = 64
GAMMA = 1.0 / F
N_LOC = N // N_CORES        # 2048 queries per core
N_TILES = N_LOC // 128      # 16 i-tiles of 128 queries
W = 2048                    # j-window: 4 PSUM banks
NW = M // W                 # 4 windows per j sweep
MM_N = 512                  # matmul moving free dim (1 PSUM bank)

QW = (2048, 224, 2048, 224)  # per-window ACT cols (w0,w2 all-ACT)
CH = 1824                   # DVE cols per mixed window (w1, w3)
H1 = CH // 2                # 912
H2 = CH // 4                # 456
D_DVE = 2 * CH              # 3648 DVE cols per sweep
NP_ACT = QW[0] + QW[1]      # 2272 ACT positives (windows 0,1)
NN_ACT = QW[2] + QW[3]      # 2272 ACT negatives (windows 2,3)

# Schraudolph constants (fp16 code format): v = round(1024*(log2e*e + C -
# SIGMA)); the uint16 pattern read as fp16 is ~exp(e)*2^(C-15). SIGMA
# tuned for zero mean error under round-to-nearest (HW-verified rint).
SIGMA = float(os.environ.get("BASS_SIGMA", "0.0575"))
C16 = 23.0
A_SC = 1024.0 * np.log2(np.e)
B_SC = 1024.0 * (C16 - SIGMA)
CR_SCALE = 2.0 ** (15 - C16)
NEG_SHIFT = 32768.0 / A_SC  # jterm shift that sets the code sign bit

BF16 = mybir.dt.bfloat16
FP16 = mybir.dt.float16
F32 = mybir.dt.float32
U16 = mybir.dt.uint16
FP8 = mybir.dt.float8e4
bf16 = ml_dtypes.bfloat16
f8 = ml_dtypes.float8_e4m3fn

_compiled_cache = {}


def _build_common(nc, tc, cpool):
    x8_d = nc.dram_tensor("x8", [128, 2, N_LOC], FP8, kind="ExternalInput")
    s8_d = nc.dram_tensor("s8", [128, 2, M], FP8, kind="ExternalInput")
    u_d = nc.dram_tensor("u", [128, N_TILES], F32, kind="ExternalInput")
    out_d = nc.dram_tensor("out", [128, N_TILES], F32, kind="ExternalOutput")

    warm_act = cpool.tile([128, 1], F32)
    nc.gpsimd.memset(warm_act[:], 0.0)
    nc.scalar.activation(warm_act[:], warm_act[:], mybir.ActivationFunctionType.Exp)

    # head loads split across both HWDGE queues (Sync + Scalar, idle at
    # start) with the first window chunked so the first matmuls fire early
    x8_sb = cpool.tile([128, 2, N_LOC], FP8)
    nc.scalar.dma_start(x8_sb[:, :, 0:128], x8_d.ap()[:, :, 0:128])
    s8_sb = cpool.tile([128, 2, M], FP8)
    nc.sync.dma_start(s8_sb[:, :, 0:1024], s8_d.ap()[:, :, 0:1024])
    nc.scalar.dma_start(s8_sb[:, :, 1024:W], s8_d.ap()[:, :, 1024:W])
    u_sb = cpool.tile([128, N_TILES], F32)
    nc.scalar.dma_start(u_sb[:], u_d.ap()[:])
    for w in range(1, NW):
        nc.sync.dma_start(
            s8_sb[:, :, w * W : (w + 1) * W],
            s8_d.ap()[:, :, w * W : (w + 1) * W],
        )
    nc.sync.dma_start(x8_sb[:, :, 128:], x8_d.ap()[:, :, 128:])
    return x8_sb, s8_sb, u_sb, out_d


def _mm_window(nc, t, ps_tile, w, x8_sb, s8_sb):
    DR = mybir.MatmulPerfMode.DoubleRow
    for c in range(W // MM_N):
        nc.tensor.matmul(
            ps_tile[:, c * MM_N : (c + 1) * MM_N],
            x8_sb[:, :, t * 128 : (t + 1) * 128],
            s8_sb[:, :, w * W + c * MM_N : w * W + (c + 1) * MM_N],
            start=True,
            stop=True,
            perf_mode=DR,
        )


def _build_v10():
    nc = bacc.Bacc(
        "TRN2",
        target_bir_lowering=False,
        debug=False,
        enable_asserts=False,
        num_devices=N_CORES,
    )
    Exp = mybir.ActivationFunctionType.Exp
    mult = mybir.AluOpType.mult
    add = mybir.AluOpType.add
    subtract = mybir.AluOpType.subtract

    with tile.TileContext(nc) as tc:
        with (
            tc.tile_pool(name="const", bufs=1) as cpool,
            tc.tile_pool(name="acc", bufs=8) as apool,
            tc.tile_pool(name="stg", bufs=4) as spool,
            tc.tile_pool(name="fin", bufs=8) as fpool,
            tc.tile_pool(name="tree", bufs=2) as tpool,
            tc.tile_pool(name="psum", bufs=2, space="PSUM") as ppool,
        ):
            x8_sb, s8_sb, u_sb, out_d = _build_common(nc, tc, cpool)
            outT_sb = cpool.tile([128, N_TILES], F32)
            dvout = cpool.tile([128, 2 * H2], FP16)
            # throwaway ACT output (in-place PSUM writes would create false
            # write-vs-read ordering against the DVE's PSUM reads)
            trash = cpool.tile([128, W], FP16)

            live = {}  # t -> (acc, f2all); CR+finish deferred one i-tile

            def fold_finish(tp):
                acc, f2all = live.pop(tp)
                # low priority: the CR/STT must never outrank the next
                # tile's Schraudolph pieces in the Vector queue
                with tc.high_priority(offset=-(1 << 20)):
                    nc.vector.tensor_scalar(
                        dvout[:],
                        f2all[:],
                        CR_SCALE,
                        0.0,
                        mult,
                        add,
                        accum_out=acc[:, 4:5],
                    )
                # out = u * (((P0 + P1) - (N2 + N3)) + dve)
                f0 = fpool.tile([128, 3], F32, tag="fin")
                nc.gpsimd.tensor_tensor(f0[:, 0:1], acc[:, 0:1], acc[:, 1:2], add)
                nc.gpsimd.tensor_tensor(f0[:, 1:2], acc[:, 2:3], acc[:, 3:4], add)
                nc.gpsimd.tensor_tensor(f0[:, 2:3], f0[:, 0:1], f0[:, 1:2], subtract)
                with tc.high_priority(offset=-(1 << 20)):
                    nc.vector.scalar_tensor_tensor(
                        outT_sb[:, tp : tp + 1],
                        f0[:, 2:3],
                        acc[:, 4:5],
                        u_sb[:, tp : tp + 1],
                        add,
                        mult,
                    )

            for t in range(N_TILES):
                acc = apool.tile([128, 5], F32, tag="acc")
                stg = spool.tile([128, D_DVE], U16, tag="stg")
                f2all = tpool.tile([128, 2 * H2], FP16, tag="f2")
                live[t] = (acc, f2all)
                for w in range(NW):
                    ps_tile = ppool.tile([128, W], F32, tag="E")
                    _mm_window(nc, t, ps_tile, w, x8_sb, s8_sb)
                    qw = QW[w]
                    # ACT piece [0, qw): sign-pure by construction
                    nc.scalar.activation(
                        trash[:, 0:qw],
                        ps_tile[:, 0:qw],
                        Exp,
                        accum_out=acc[:, w : w + 1],
                    )
                    if qw == W:
                        continue
                    # DVE (Schraudolph) piece for the mixed windows, signs
                    # baked into the matmul jterm
                    ci = 0 if w == 1 else 1
                    nc.vector.tensor_scalar(
                        stg[:, ci * CH : (ci + 1) * CH],
                        ps_tile[:, qw:W],
                        A_SC,
                        B_SC,
                        mult,
                        add,
                    )
                    # 4x fold of this window's codes -> f2all chunk
                    ch = stg[:, ci * CH : (ci + 1) * CH].bitcast(FP16)
                    if w == 1:
                        fa = tpool.tile([128, H1], FP16, tag="t1")
                        nc.gpsimd.tensor_tensor(fa[:], ch[:, 0:H1], ch[:, H1:CH], add)
                        nc.gpsimd.tensor_tensor(
                            f2all[:, 0:H2], fa[:, 0:H2], fa[:, H2:H1], add
                        )
                    else:
                        fa = tpool.tile([128, H1], FP16, tag="t3")
                        with tc.high_priority(offset=-(1 << 20)):
                            nc.vector.tensor_tensor(
                                fa[:], ch[:, 0:H1], ch[:, H1:CH], add
                            )
                        nc.gpsimd.tensor_tensor(
                            f2all[:, H2 : 2 * H2], fa[:, 0:H2], fa[:, H2:H1], add
                        )
                if t >= 1:
                    fold_finish(t - 1)
            fold_finish(N_TILES - 1)

            nc.sync.dma_start(out_d.ap()[:], outT_sb[:])

    nc.compile()
    return nc


def _f8(v):
    return v.astype(f8)


def _prepare(x, supports, alphas):
    x = np.asarray(x, dtype=np.float32)
    supports = np.asarray(supports, dtype=np.float32)
    alphas = np.asarray(alphas, dtype=np.float32)

    a64 = alphas.astype(np.float64)
    s64 = supports.astype(np.float64)
    jterm = -GAMMA * (s64 * s64).sum(axis=1) + np.maximum(
        np.log(np.maximum(np.abs(a64), 1e-300)), -11.0
    )

    order = np.argsort(np.abs(a64), kind="stable")
    allP = order[a64[order] > 0]
    allN = order[a64[order] <= 0]
    assert len(allP) >= NP_ACT and len(allN) >= NN_ACT, (len(allP), len(allN))
    act_P = allP[-NP_ACT:]          # windows 0,1 ACT pieces (largest |a|)
    act_N = allN[-NN_ACT:]          # windows 2,3 ACT pieces (largest |a|)
    dve_N = allN[:-NN_ACT]
    dve_P = allP[:-NP_ACT]
    dve_seq = np.concatenate([dve_N, dve_P])
    assert len(dve_seq) == D_DVE

    # negative-alpha DVE columns: shift jterm so the Schraudolph code gets
    # the fp16 sign bit (code += 32768)
    jterm = jterm.copy()
    jterm[dve_N] += NEG_SHIFT
    # code-range safety: sign bit must survive the worst-case x.s swing
    assert (jterm[dve_N].min() - 1.8) * A_SC + B_SC > 32768.0 + 200.0
    assert (jterm[dve_seq].max() + 1.8) * A_SC + B_SC < 65535.0 - 200.0

    # column permutation: window w = [ACT block (Q), DVE block (DW)]
    perm = np.empty(M, dtype=np.int64)
    for w in range(NW):
        base = w * W
        qw = QW[w]
        if w == 0:
            perm[base : base + qw] = act_P[-qw:]
        elif w == 1:
            perm[base : base + qw] = act_P[:qw]
        elif w == 2:
            perm[base : base + qw] = act_N[-qw:]
        else:
            perm[base : base + qw] = act_N[:qw]
        if qw < W:
            ci = 0 if w == 1 else 1
            perm[base + qw : base + W] = dve_seq[ci * CH : (ci + 1) * CH]

    # fp8 range-scaled hi/lo splits
    xs4 = (x.T / 4.0).astype(np.float64)
    sp8 = (supports[perm].T / 8.0).astype(np.float64)
    A1 = _f8(xs4)
    A2 = _f8(16.0 * (xs4 - A1.astype(np.float64)))
    A1o16 = _f8(A1.astype(np.float64) / 16.0)
    B1 = _f8(sp8)
    B2 = _f8(16.0 * (sp8 - B1.astype(np.float64)))
    B1o16 = _f8(B1.astype(np.float64) / 16.0)
    jt = jterm[perm]
    J1 = _f8(jt)
    J2 = _f8(16.0 * (jt - J1.astype(np.float64)))
    J3 = _f8(256.0 * (jt - J1.astype(np.float64) - J2.astype(np.float64) / 16.0))

    xrows = np.zeros((256, N), dtype=f8)
    srows = np.zeros((256, M), dtype=f8)
    xrows[0:64] = A1
    srows[0:64] = B1
    xrows[64:128] = A1o16
    srows[64:128] = B2
    xrows[128:192] = A2
    srows[128:192] = B1o16
    xrows[192] = f8(1.0)
    srows[192] = J1
    xrows[193] = f8(0.0625)
    srows[193] = J2
    xrows[194] = f8(0.00390625)
    srows[194] = J3
    x8 = xrows.reshape(128, 2, N)
    s8 = srows.reshape(128, 2, M)

    u = np.exp(-GAMMA * (x.astype(np.float64) ** 2).sum(axis=1)).astype(np.float32)

    in_maps = []
    for c in range(N_CORES):
        sl = slice(c * N_LOC, (c + 1) * N_LOC)
        in_maps.append(
            {
                "x8": np.ascontiguousarray(x8[:, :, sl]),
                "s8": s8,
                "u": np.ascontiguousarray(u[sl].reshape(N_TILES, 128).T),
            }
        )
    return in_maps


def _run(x, supports, alphas, trace=False, **run_kwargs):
    in_maps = _prepare(x, supports, alphas)
    key = (QW, SIGMA)
    if key not in _compiled_cache:
        _compiled_cache[key] = _build_v10()
    nc = _compiled_cache[key]
    res = run_bass_kernel_spmd(
        nc, in_maps, core_ids=list(range(N_CORES)), trace=trace, **run_kwargs
    )
    outs = [r["out"].T.reshape(-1) for r in res.results]
    return np.concatenate(outs).astype(np.float32), res


def kernel(x, supports, alphas):
    out, _ = _run(x, supports, alphas, trace=False)
    return out


# revision 28
# speedup vs baseline: 1.1893x; 1.1893x over previous
"""RBF-kernel SVM decision function on 8 TRN2 NeuronCores.

out[i] = sum_j alphas[j] * exp(-GAMMA * ||x[i] - supports[j]||^2)

Factorization: out_i = u_i * sum_j sgn_j * exp(e_ij),
  e_ij = (x_i/32).s_j + (ln|a_j| - g|s_j|^2)   [PSUM via one fp8 DoubleRow
         matmul; see row scheme in _prepare]
  u_i  = exp(-g|x_i|^2)                        [applied at the end, per i]

PE: fp8 e4m3 DoubleRow matmul built from range-scaled hi/lo splits
(A1=e4m3(x/4), A2=e4m3(16(x/4-A1)); B1=e4m3(s/8), B2=e4m3(16(s/8-B1));
w = A1.B1 + (A1/16).B2 + A2.(B1/16), jterm via 3 rows J1+J2/16+J3/256;
196 live rows of [128 partitions, 2 sub-rows]). Exponent error ~6e-4 rms.

Drain structure: PSUM holds two 2048-col windows (ping-pong); each window
is [0:Q) for ACT and [Q:2048) for DVE:
  - ACT: one ACTIVATE(Exp, accum_out) per window; the host permutation
    makes each window's ACT piece sign-pure (windows 0,1 take the
    largest-|a| positives, windows 2,3 the largest-|a| negatives, so ACT
    covers the global top-|a| columns of both signs).
  - DVE: one Schraudolph TENSOR_SCALAR per window producing uint16 codes
    (round(1024*(log2e*e + 23 - SIGMA))); the code bit pattern read as
    fp16 is ~exp(e)*2^8. Negative-alpha DVE columns get their SIGN baked
    into the matmul: the host adds 32768/A_SC to their jterm so the code
    lands with the fp16 sign bit set -- no flip splits, one piece/window.
The codes fold 4x before a single CACHE_REDUCE: windows 0-2 fold on the
otherwise-idle Pool engine (fires per window, right after its codes
land); window 3 folds on the DVE (fp16 2x mode, short tail). The CR and
the tiny final combine are emitted one i-tile late (and at low scheduler
priority) so they never block the DVE FIFO ahead of the next tile's
pieces.

Steady state is four-way balanced: Scalar ~7.0us/i-tile (4 ACTIVATE +
4 READ_ACCUMULATOR), Vector ~6.9 (4 TS + CR + w3 folds), Pool ~6.7
(6 folds + combine), PE ~6.6 (16 DoubleRow MMs; per-MM LDWEIGHTS is
compiler-mandated). Further Q rebalancing moves time between Scalar and
Vector symmetrically -- Q=1136 sits at the crossover.
"""

import os
import sys

for p in ("/opt/trn_rl_repo",):
    if p not in sys.path:
        sys.path.insert(0, p)

import numpy as np
import ml_dtypes

import concourse.bass as bass
import concourse.tile as tile
from concourse import bacc, mybir
from concourse.bass_utils import run_bass_kernel_spmd

N_CORES = 8
N = 16384
M = 8192
F = 64
GAMMA = 1.0 / F
N_LOC = N // N_CORES        # 2048 queries per core
N_TILES = N_LOC // 128      # 16 i-tiles of 128 queries
W = 2048                    # j-window: 4 PSUM banks
NW = M // W                 # 4 windows per j sweep
MM_N = 512                  # matmul moving free dim (1 PSUM bank)

Q = 1136                    # ACT cols per window (rest -> DVE)
DW = W - Q                  # 912 DVE cols per window
H1 = DW // 2                # 456
H2 = DW // 4                # 228
D_DVE = NW * DW             # 3648 DVE cols per sweep
NP_ACT = 2 * Q              # ACT positives (windows 0,1)
NN_ACT = 2 * Q              # ACT negatives (windows 2,3)

# Schraudolph constants (fp16 code format): v = round(1024*(log2e*e + C -
# SIGMA)); the uint16 pattern read as fp16 is ~exp(e)*2^(C-15). SIGMA
# tuned for zero mean error under round-to-nearest (HW-verified rint).
SIGMA = float(os.environ.get("BASS_SIGMA", "0.0575"))
C16 = 23.0
A_SC = 1024.0 * np.log2(np.e)
B_SC = 1024.0 * (C16 - SIGMA)
CR_SCALE = 2.0 ** (15 - C16)
NEG_SHIFT = 32768.0 / A_SC  # jterm shift that sets the code sign bit

BF16 = mybir.dt.bfloat16
FP16 = mybir.dt.float16
F32 = mybir.dt.float32
U16 = mybir.dt.uint16
FP8 = mybir.dt.float8e4
bf16 = ml_dtypes.bfloat16
f8 = ml_dtypes.float8_e4m3fn

_compiled_cache = {}


def _build_common(nc, tc, cpool):
    x8_d = nc.dram_tensor("x8", [128, 2, N_LOC], FP8, kind="ExternalInput")
    s8_d = nc.dram_tensor("s8", [128, 2, M], FP8, kind="ExternalInput")
    u_d = nc.dram_tensor("u", [128, N_TILES], F32, kind="ExternalInput")
    out_d = nc.dram_tensor("out", [128, N_TILES], F32, kind="ExternalOutput")

    warm_act = cpool.tile([128, 1], F32)
    nc.gpsimd.memset(warm_act[:], 0.0)
    nc.scalar.activation(warm_act[:], warm_act[:], mybir.ActivationFunctionType.Exp)

    # head loads split across both HWDGE queues (Sync + Scalar, idle at
    # start) with the first window chunked so the first matmuls fire early
    x8_sb = cpool.tile([128, 2, N_LOC], FP8)
    nc.scalar.dma_start(x8_sb[:, :, 0:128], x8_d.ap()[:, :, 0:128])
    s8_sb = cpool.tile([128, 2, M], FP8)
    nc.sync.dma_start(s8_sb[:, :, 0:1024], s8_d.ap()[:, :, 0:1024])
    nc.scalar.dma_start(s8_sb[:, :, 1024:W], s8_d.ap()[:, :, 1024:W])
    u_sb = cpool.tile([128, N_TILES], F32)
    nc.scalar.dma_start(u_sb[:], u_d.ap()[:])
    for w in range(1, NW):
        nc.sync.dma_start(
            s8_sb[:, :, w * W : (w + 1) * W],
            s8_d.ap()[:, :, w * W : (w + 1) * W],
        )
    nc.sync.dma_start(x8_sb[:, :, 128:], x8_d.ap()[:, :, 128:])
    return x8_sb, s8_sb, u_sb, out_d


def _mm_window(nc, t, ps_tile, w, x8_sb, s8_sb):
    DR = mybir.MatmulPerfMode.DoubleRow
    for c in range(W // MM_N):
        nc.tensor.matmul(
            ps_tile[:, c * MM_N : (c + 1) * MM_N],
            x8_sb[:, :, t * 128 : (t + 1) * 128],
            s8_sb[:, :, w * W + c * MM_N : w * W + (c + 1) * MM_N],
            start=True,
            stop=True,
            perf_mode=DR,
        )


def _build_v10():
    nc = bacc.Bacc(
        "TRN2",
        target_bir_lowering=False,
        debug=False,
        enable_asserts=False,
        num_devices=N_CORES,
    )
    Exp = mybir.ActivationFunctionType.Exp
    mult = mybir.AluOpType.mult
    add = mybir.AluOpType.add
    subtract = mybir.AluOpType.subtract

    with tile.TileContext(nc) as tc:
        with (
            tc.tile_pool(name="const", bufs=1) as cpool,
            tc.tile_pool(name="acc", bufs=8) as apool,
            tc.tile_pool(name="stg", bufs=4) as spool,
            tc.tile_pool(name="fin", bufs=8) as fpool,
            tc.tile_pool(name="tree", bufs=2) as tpool,
            tc.tile_pool(name="psum", bufs=2, space="PSUM") as ppool,
        ):
            x8_sb, s8_sb, u_sb, out_d = _build_common(nc, tc, cpool)
            outT_sb = cpool.tile([128, N_TILES], F32)
            dvout = cpool.tile([128, DW], FP16)
            # throwaway ACT output (in-place PSUM writes would create false
            # write-vs-read ordering against the DVE's PSUM reads)
            trash = cpool.tile([128, Q], FP16)

            live = {}  # t -> (acc, f2all); CR+finish deferred one i-tile

            def fold_finish(tp):
                acc, f2all = live.pop(tp)
                # low priority: the CR/STT must never outrank the next
                # tile's Schraudolph pieces in the Vector queue
                with tc.high_priority(offset=-(1 << 20)):
                    nc.vector.tensor_scalar(
                        dvout[:],
                        f2all[:],
                        CR_SCALE,
                        0.0,
                        mult,
                        add,
                        accum_out=acc[:, 4:5],
                    )
                # out = u * (((P0 + P1) - (N2 + N3)) + dve)
                f0 = fpool.tile([128, 3], F32, tag="fin")
                nc.gpsimd.tensor_tensor(f0[:, 0:1], acc[:, 0:1], acc[:, 1:2], add)
                nc.gpsimd.tensor_tensor(f0[:, 1:2], acc[:, 2:3], acc[:, 3:4], add)
                nc.gpsimd.tensor_tensor(f0[:, 2:3], f0[:, 0:1], f0[:, 1:2], subtract)
                with tc.high_priority(offset=-(1 << 20)):
                    nc.vector.scalar_tensor_tensor(
                        outT_sb[:, tp : tp + 1],
                        f0[:, 2:3],
                        acc[:, 4:5],
                        u_sb[:, tp : tp + 1],
                        add,
                        mult,
                    )

            for t in range(N_TILES):
                acc = apool.tile([128, 5], F32, tag="acc")
                stg = spool.tile([128, D_DVE], U16, tag="stg")
                f2all = tpool.tile([128, DW], FP16, tag="f2")
                live[t] = (acc, f2all)
                for w in range(NW):
                    ps_tile = ppool.tile([128, W], F32, tag="E")
                    _mm_window(nc, t, ps_tile, w, x8_sb, s8_sb)
                    # ACT piece [0, Q): sign-pure by construction
                    nc.scalar.activation(
                        trash[:],
                        ps_tile[:, 0:Q],
                        Exp,
                        accum_out=acc[:, w : w + 1],
                    )
                    # DVE (Schraudolph) piece: one per window, signs baked
                    # into the matmul jterm
                    nc.vector.tensor_scalar(
                        stg[:, w * DW : (w + 1) * DW],
                        ps_tile[:, Q:W],
                        A_SC,
                        B_SC,
                        mult,
                        add,
                    )
                    # 4x fold of this window's codes -> f2all chunk
                    ch = stg[:, w * DW : (w + 1) * DW].bitcast(FP16)
                    if w < 3:
                        fa = tpool.tile([128, H1], FP16, tag="t1")
                        nc.gpsimd.tensor_tensor(fa[:], ch[:, 0:H1], ch[:, H1:DW], add)
                        nc.gpsimd.tensor_tensor(
                            f2all[:, w * H2 : (w + 1) * H2],
                            fa[:, 0:H2],
                            fa[:, H2:H1],
                            add,
                        )
                    else:
                        fa = tpool.tile([128, H1], FP16, tag="t3")
                        with tc.high_priority(offset=-(1 << 20)):
                            nc.vector.tensor_tensor(
                                fa[:], ch[:, 0:H1], ch[:, H1:DW], add
                            )
                        nc.gpsimd.tensor_tensor(
                            f2all[:, 3 * H2 : DW], fa[:, 0:H2], fa[:, H2:H1], add
                        )
                if t >= 1:
                    fold_finish(t - 1)
            fold_finish(N_TILES - 1)

            nc.sync.dma_start(out_d.ap()[:], outT_sb[:])

    nc.compile()
    return nc


def _f8(v):
    return v.astype(f8)


def _prepare(x, supports, alphas):
    x = np.asarray(x, dtype=np.float32)
    supports = np.asarray(supports, dtype=np.float32)
    alphas = np.asarray(alphas, dtype=np.float32)

    a64 = alphas.astype(np.float64)
    s64 = supports.astype(np.float64)
    jterm = -GAMMA * (s64 * s64).sum(axis=1) + np.maximum(
        np.log(np.maximum(np.abs(a64), 1e-300)), -11.0
    )

    order = np.argsort(np.abs(a64), kind="stable")
    allP = order[a64[order] > 0]
    allN = order[a64[order] <= 0]
    assert len(allP) >= NP_ACT and len(allN) >= NN_ACT, (len(allP), len(allN))
    act_P = allP[-NP_ACT:]          # windows 0,1 ACT pieces (largest |a|)
    act_N = allN[-NN_ACT:]          # windows 2,3 ACT pieces (largest |a|)
    dve_N = allN[:-NN_ACT]
    dve_P = allP[:-NP_ACT]
    dve_seq = np.concatenate([dve_N, dve_P])
    assert len(dve_seq) == D_DVE

    # negative-alpha DVE columns: shift jterm so the Schraudolph code gets
    # the fp16 sign bit (code += 32768)
    jterm = jterm.copy()
    jterm[dve_N] += NEG_SHIFT
    # code-range safety: sign bit must survive the worst-case x.s swing
    assert (jterm[dve_N].min() - 1.8) * A_SC + B_SC > 32768.0 + 200.0
    assert (jterm[dve_seq].max() + 1.8) * A_SC + B_SC < 65535.0 - 200.0

    # column permutation: window w = [ACT block (Q), DVE block (DW)]
    perm = np.empty(M, dtype=np.int64)
    for w in range(NW):
        base = w * W
        if w < 2:
            perm[base : base + Q] = act_P[w * Q : (w + 1) * Q]
        else:
            perm[base : base + Q] = act_N[(w - 2) * Q : (w - 1) * Q]
        perm[base + Q : base + W] = dve_seq[w * DW : (w + 1) * DW]

    # fp8 range-scaled hi/lo splits
    xs4 = (x.T / 4.0).astype(np.float64)
    sp8 = (supports[perm].T / 8.0).astype(np.float64)
    A1 = _f8(xs4)
    A2 = _f8(16.0 * (xs4 - A1.astype(np.float64)))
    A1o16 = _f8(A1.astype(np.float64) / 16.0)
    B1 = _f8(sp8)
    B2 = _f8(16.0 * (sp8 - B1.astype(np.float64)))
    B1o16 = _f8(B1.astype(np.float64) / 16.0)
    jt = jterm[perm]
    J1 = _f8(jt)
    J2 = _f8(16.0 * (jt - J1.astype(np.float64)))
    J3 = _f8(256.0 * (jt - J1.astype(np.float64) - J2.astype(np.float64) / 16.0))

    xrows = np.zeros((256, N), dtype=f8)
    srows = np.zeros((256, M), dtype=f8)
    xrows[0:64] = A1
    srows[0:64] = B1
    xrows[64:128] = A1o16
    srows[64:128] = B2
    xrows[128:192] = A2
    srows[128:192] = B1o16
    xrows[192] = f8(1.0)
    srows[192] = J1
    xrows[193] = f8(0.0625)
    srows[193] = J2
    xrows[194] = f8(0.00390625)
    srows[194] = J3
    x8 = xrows.reshape(128, 2, N)
    s8 = srows.reshape(128, 2, M)

    u = np.exp(-GAMMA * (x.astype(np.float64) ** 2).sum(axis=1)).astype(np.float32)

    in_maps = []
    for c in range(N_CORES):
        sl = slice(c * N_LOC, (c + 1) * N_LOC)
        in_maps.append(
            {
                "x8": np.ascontiguousarray(x8[:, :, sl]),
                "s8": s8,
                "u": np.ascontiguousarray(u[sl].reshape(N_TILES, 128).T),
            }
        )
    return in_maps


def _run(x, supports, alphas, trace=False, **run_kwargs):
    in_maps = _prepare(x, supports, alphas)
    key = (Q, SIGMA)
    if key not in _compiled_cache:
        _compiled_cache[key] = _build_v10()
    nc = _compiled_cache[key]
    res = run_bass_kernel_spmd(
        nc, in_maps, core_ids=list(range(N_CORES)), trace=trace, **run_kwargs
    )
    outs = [r["out"].T.reshape(-1) for r in res.results]
    return np.concatenate(outs).astype(np.float32), res


def kernel(x, supports, alphas):
    out, _ = _run(x, supports, alphas, trace=False)
    return out
